# revision 1
# baseline (speedup 1.0000x reference)
"""Trainium2 Bass kernel for nn_AdultConnectome (gnn_message_passing).

Computes y = A^L @ x for a COO sparse adjacency A (100000 nodes, 3.2M edges),
x [100000, 512] fp32, L = layer_number hops.

Distribution: 8 NeuronCores; core c owns the column-node block
[12544*c, 12544*(c+1)) and ALL 512 features (bf16). Edges are partitioned by
their source (col) node block, so every per-edge gather index is block-local
(< 12544, fits the gather DMA's int16 index limit). Each hop:

  1. dma_gather: per edge e, fetch h[col_local[e], :] (512 bf16 = 1KB rows)
     from the core-local table in HBM. Edges are pre-sorted by destination
     row, padded per 128-row destination block to a uniform chunk count.
     Gathers are capped at 1024 indices (larger single dma_gathers overflow
     the SWDGE descriptor ring and fault the device).
  2. For each 128-edge chunk, load the host-precomputed scatter matrix
     P[e, r] = w[e] * (row_local[e] == r) (bf16, streamed from HBM) and
     accumulate PSUM[r, f] += P^T @ G on TensorE. This is the segment-sum.
  3. Evict each 128-row block to a [100352, 512] bf16 partial in HBM.
  4. ReduceScatter(add) over all 8 cores sums the partials and hands core c
     its own 12544-row block for the next hop's gather table.

All structure (chunk counts, padding) is computed host-side from the actual
edge data and baked into the compiled graph; it is identical on all 8 cores
(SPMD), with per-core differences only in input tensors.
"""

import numpy as np
import ml_dtypes

import concourse.bass as bass
import concourse.bacc as bacc
import concourse.tile as tile
import concourse.mybir as mybir
from concourse.bass_utils import run_bass_kernel_spmd

BF16 = ml_dtypes.bfloat16

N_CORES = 8
P = 128
N_NODES = 100000
N_FEAT = 512
NB = 12544                 # nodes per core block (100352 = 8 * 12544)
NPAD = NB * N_CORES        # 100352
NRB = NPAD // P            # 784 row blocks
GCH = 8                    # chunks (128 edges each) per gather (1024 idx max)
NSEG = 7                   # ReduceScatter slabs per hop (98 = 7*14)


def _prep_core(rows, cols, ws, core):
    """Per-core edge preprocessing."""
    lo, hi = NB * core, NB * (core + 1)
    m = (cols >= lo) & (cols < hi)
    r = rows[m]
    c = (cols[m] - lo).astype(np.int64)
    w = ws[m]
    order = np.argsort(r, kind="stable")
    r, c, w = r[order], c[order], w[order]
    rb = r >> 7
    rl = (r & 127).astype(np.int64)
    cnt = np.bincount(rb, minlength=NRB)
    return r, c, w, rb, rl, cnt


def _block_seq():
    """Row-block processing order: segment-major (q, core, i) so that each
    of the NSEG ReduceScatter slabs covers a contiguous run of processed
    blocks and can be issued while later segments still compute."""
    bpc = NRB // N_CORES              # 98 blocks per core block
    bps = bpc // NSEG                 # 14 blocks per (segment, core)
    seq = []
    for q in range(NSEG):
        for cc in range(N_CORES):
            for i in range(bps):
                seq.append(cc * bpc + q * bps + i)
    return np.array(seq, dtype=np.int64)


def _pack_core(r, c, w, rb, rl, cnt, nchunk_pb):
    """Pack one core's edges into padded device arrays (idx + P tiles)."""
    epb = nchunk_pb * P               # padded edges per row block
    tot = NRB * epb
    ncht = NRB * nchunk_pb
    bs = _block_seq()
    posof = np.empty(NRB, dtype=np.int64)
    posof[bs] = np.arange(NRB)
    order2 = np.argsort(posof[rb], kind="stable")
    r, c, w, rb, rl = r[order2], c[order2], w[order2], rb[order2], rl[order2]
    pos = posof[rb]
    cnt_seq = cnt[bs]
    col_pad = np.zeros(tot, dtype=np.int16)
    starts = np.zeros(NRB, dtype=np.int64)
    starts[1:] = np.cumsum(cnt_seq)[:-1]
    j_within = np.arange(len(r)) - starts[pos]
    slot = pos * epb + j_within
    col_pad[slot] = c.astype(np.int16)

    # P tiles: logically [ncht, 128, 128] bf16; P[k, p, rl] = w for edge
    # (k*128+p). Device layout groups GCH chunks per gather with partition-
    # major rows so one plain 2D DMA per gather lands them in SBUF:
    # [ngath*128, GCH*128] where row (gi*128+p), cols [q*128:(q+1)*128] is
    # chunk (gi*GCH+q)'s partition-p row.
    p_tiles = np.zeros(ncht * P * P, dtype=BF16)
    chunk = slot // P
    part = slot % P
    p_tiles[chunk * (P * P) + part * P + rl] = w.astype(BF16)
    ngath_p = ncht // GCH
    p_tiles = (p_tiles.reshape(ngath_p, GCH, P, P)
               .transpose(0, 2, 1, 3).reshape(ngath_p * P, GCH * P))

    # gather idx layout: per gather of GCH chunks, wrapped [16, n/16],
    # tiled to 128 partitions; gathers concatenated along free dim
    gpg = GCH * P
    ngath = tot // gpg
    idx_grp = col_pad.reshape(ngath, gpg // 16, 16)
    idx_wrapped = idx_grp.transpose(0, 2, 1)
    idx_dev = np.tile(idx_wrapped, (1, 8, 1))
    idx_dev = np.concatenate(idx_dev, axis=1)
    return {
        "gidx": np.ascontiguousarray(idx_dev),
        "ptiles": np.ascontiguousarray(p_tiles),
    }


def _build_graph(n_hops, nchunk_pb):
    """Build the SPMD Bass graph (identical for all cores)."""
    ncht = NRB * nchunk_pb
    gpg = GCH * P
    nidx_tot = NRB * nchunk_pb * P
    ngath = nidx_tot // gpg
    assert ncht % GCH == 0

    nc = bacc.Bacc("TRN2", target_bir_lowering=False, debug=False,
                   num_devices=N_CORES)

    h0_in = nc.dram_tensor("h0", [NB, N_FEAT], mybir.dt.bfloat16,
                           kind="ExternalInput")
    gidx_in = nc.dram_tensor("gidx", [P, nidx_tot // 16], mybir.dt.int16,
                             kind="ExternalInput")
    pt_in = nc.dram_tensor("ptiles", [ngath * P, GCH * P], mybir.dt.bfloat16,
                           kind="ExternalInput")
    y_out = nc.dram_tensor("y", [NB, N_FEAT], mybir.dt.bfloat16,
                           kind="ExternalOutput")

    with tile.TileContext(nc) as tc:
        with tc.tile_pool(name="sbuf", bufs=8) as sbuf, \
             tc.tile_pool(name="sbuf_idx", bufs=1) as sbuf_idx, \
             tc.tile_pool(name="psum", bufs=8, space="PSUM") as psum, \
             tc.tile_pool(name="dram", bufs=2, space="DRAM") as dram:

            h_tabs = [h0_in.ap()[:, :]]
            for hop in range(n_hops):
                partial = dram.tile([NPAD, N_FEAT], mybir.dt.bfloat16,
                                    tag="partial")
                h_tab = h_tabs[hop]
                idx_hop = sbuf_idx.tile([P, nidx_tot // 16], mybir.dt.int16,
                                        tag="idxhop")
                nc.sync.dma_start(idx_hop[:], gidx_in.ap()[:, :])
                g_list = [None] * ngath
                p_list = [None] * ngath

                def issue_gather(gi, h_tab=h_tab, g_list=g_list,
                                 p_list=p_list, idx_hop=idx_hop):
                    g_t = sbuf.tile([P, GCH, N_FEAT],
                                    mybir.dt.bfloat16, tag="gath")
                    nc.gpsimd.dma_gather(
                        out_ap=g_t[:],
                        in_ap=h_tab,
                        idxs_ap=idx_hop[:, gi * (gpg // 16):(gi + 1) * (gpg // 16)],
                        num_idxs=gpg,
                        num_idxs_reg=gpg,
                        elem_size=N_FEAT,
                    )
                    g_list[gi] = g_t
                    # P tiles for this gather's GCH chunks, loaded via the
                    # ScalarE HWDGE ring so they don't queue behind the
                    # Sync-ring evict/idx DMAs
                    p_t = sbuf.tile([P, GCH, P], mybir.dt.bfloat16,
                                    tag="ptile")
                    nc.scalar.dma_start(
                        p_t[:], pt_in.ap()[gi * P:(gi + 1) * P, :])
                    p_list[gi] = p_t

                h_next = dram.tile([NB, N_FEAT], mybir.dt.bfloat16,
                                   tag="hnext")
                bpseg = NRB // NSEG        # 112 processed blocks per slab
                rseg = NB // NSEG          # 1792 h_next rows per slab
                rs_next = 0

                def maybe_issue_rs(done_blocks, force=False):
                    nonlocal rs_next
                    while rs_next < NSEG:
                        need = (rs_next + 1) * bpseg + 128
                        if not force and done_blocks < min(need, NRB):
                            break
                        if not force and rs_next == NSEG - 1:
                            break
                        j = rs_next
                        nc.gpsimd.collective_compute(
                            "ReduceScatter",
                            mybir.AluOpType.add,
                            replica_groups=[list(range(N_CORES))],
                            ins=[partial[j * bpseg * P:(j + 1) * bpseg * P,
                                         :].opt()],
                            outs=[h_next[j * rseg:(j + 1) * rseg, :].opt()],
                        )
                        rs_next += 1

                next_rb = 0
                for gi in range(ngath):
                    issue_gather(gi)
                    while (next_rb < NRB
                           and (next_rb + 1) * nchunk_pb <= (gi + 1) * GCH):
                        rbid = next_rb
                        ps = psum.tile([P, N_FEAT], mybir.dt.float32,
                                       space="PSUM", tag="ps")
                        for cch in range(nchunk_pb):
                            k = rbid * nchunk_pb + cch
                            nc.tensor.matmul(
                                out=ps[:],
                                lhsT=p_list[k // GCH][:, k % GCH, :],
                                rhs=g_list[k // GCH][:, k % GCH, :],
                                start=(cch == 0),
                                stop=(cch == nchunk_pb - 1),
                            )
                        ev = sbuf.tile([P, N_FEAT], mybir.dt.bfloat16,
                                       tag="evict")
                        nc.vector.tensor_copy(ev[:], ps[:])
                        nc.sync.dma_start(
                            partial[rbid * P:(rbid + 1) * P, :], ev[:])
                        next_rb += 1
                        maybe_issue_rs(next_rb)
                maybe_issue_rs(NRB, force=True)
                h_tabs.append(h_next[:])

            nc.sync.dma_start(y_out.ap()[:, :], h_tabs[n_hops])

    nc.compile()
    return nc


_GRAPH_CACHE = {}


def kernel(x, weights, row, col, layer_number):
    x = np.asarray(x)
    weights = np.asarray(weights)
    rows = np.asarray(row).astype(np.int64)
    cols = np.asarray(col).astype(np.int64)
    n_hops = int(layer_number)
    if n_hops == 0:
        return x.astype(np.float32)

    preps = [_prep_core(rows, cols, weights, c) for c in range(N_CORES)]
    nchunk_pb = max(int(np.ceil(p[5].max() / P)) for p in preps)
    nchunk_pb = max(nchunk_pb, 1)

    key = (n_hops, nchunk_pb)
    if key not in _GRAPH_CACHE:
        _GRAPH_CACHE[key] = _build_graph(n_hops, nchunk_pb)
    nc = _GRAPH_CACHE[key]

    x_pad = np.zeros((NPAD, N_FEAT), dtype=np.float32)
    x_pad[:N_NODES] = x
    x_bf = x_pad.astype(BF16)

    in_maps = []
    for c in range(N_CORES):
        dev = _pack_core(*preps[c], nchunk_pb)
        in_maps.append({
            "h0": np.ascontiguousarray(x_bf[NB * c:NB * (c + 1)]),
            "gidx": dev["gidx"],
            "ptiles": dev["ptiles"],
        })

    res = run_bass_kernel_spmd(nc, in_maps, core_ids=list(range(N_CORES)))
    y = np.concatenate([res.results[c]["y"].astype(np.float32)
                        for c in range(N_CORES)], axis=0)
    return y[:N_NODES]



# revision 3
# speedup vs baseline: 1.0026x; 1.0026x over previous
"""Trainium2 Bass kernel for nn_AdultConnectome (gnn_message_passing).

Computes y = A^L @ x for a COO sparse adjacency A (100000 nodes, 3.2M edges),
x [100000, 512] fp32, L = layer_number hops.

Distribution: 8 NeuronCores; core c owns the column-node block
[12544*c, 12544*(c+1)) and ALL 512 features (bf16). Edges are partitioned by
their source (col) node block, so every per-edge gather index is block-local
(< 12544, fits the gather DMA's int16 index limit). Each hop:

  1. dma_gather: per edge e, fetch h[col_local[e], :] (512 bf16 = 1KB rows)
     from the core-local table in HBM. Edges are pre-sorted by destination
     row, padded per 128-row destination block to a uniform chunk count.
     Gathers are capped at 1024 indices (larger single dma_gathers overflow
     the SWDGE descriptor ring and fault the device).
  2. For each 128-edge chunk, load the host-precomputed scatter matrix
     P[e, r] = w[e] * (row_local[e] == r) (bf16, streamed from HBM) and
     accumulate PSUM[r, f] += P^T @ G on TensorE. This is the segment-sum.
  3. Evict each 128-row block to a [100352, 512] bf16 partial in HBM.
  4. ReduceScatter(add) over all 8 cores sums the partials and hands core c
     its own 12544-row block for the next hop's gather table.

All structure (chunk counts, padding) is computed host-side from the actual
edge data and baked into the compiled graph; it is identical on all 8 cores
(SPMD), with per-core differences only in input tensors.
"""

import numpy as np
import ml_dtypes

import concourse.bass as bass
import concourse.bacc as bacc
import concourse.tile as tile
import concourse.mybir as mybir
from concourse.bass_utils import run_bass_kernel_spmd

BF16 = ml_dtypes.bfloat16

N_CORES = 8
P = 128
N_NODES = 100000
N_FEAT = 512
NB = 12544                 # nodes per core block (100352 = 8 * 12544)
NPAD = NB * N_CORES        # 100352
NRB = NPAD // P            # 784 row blocks
GCH = 8                    # chunks (128 edges each) per gather (1024 idx max)
NSEG = 7                   # ReduceScatter slabs per hop (98 = 7*14)


def _prep_core(rows, cols, ws, core):
    """Per-core edge preprocessing."""
    lo, hi = NB * core, NB * (core + 1)
    m = (cols >= lo) & (cols < hi)
    r = rows[m]
    c = (cols[m] - lo).astype(np.int64)
    w = ws[m]
    order = np.argsort(r, kind="stable")
    r, c, w = r[order], c[order], w[order]
    rb = r >> 7
    rl = (r & 127).astype(np.int64)
    cnt = np.bincount(rb, minlength=NRB)
    return r, c, w, rb, rl, cnt


def _block_seq():
    """Row-block processing order: segment-major (q, core, i) so that each
    of the NSEG ReduceScatter slabs covers a contiguous run of processed
    blocks and can be issued while later segments still compute."""
    bpc = NRB // N_CORES              # 98 blocks per core block
    bps = bpc // NSEG                 # 14 blocks per (segment, core)
    seq = []
    for q in range(NSEG):
        for cc in range(N_CORES):
            for i in range(bps):
                seq.append(cc * bpc + q * bps + i)
    return np.array(seq, dtype=np.int64)


def _pack_core(r, c, w, rb, rl, cnt, nchunk_pb):
    """Pack one core's edges into padded device arrays (idx + P tiles)."""
    epb = nchunk_pb * P               # padded edges per row block
    tot = NRB * epb
    ncht = NRB * nchunk_pb
    bs = _block_seq()
    posof = np.empty(NRB, dtype=np.int64)
    posof[bs] = np.arange(NRB)
    order2 = np.argsort(posof[rb], kind="stable")
    r, c, w, rb, rl = r[order2], c[order2], w[order2], rb[order2], rl[order2]
    pos = posof[rb]
    cnt_seq = cnt[bs]
    col_pad = np.zeros(tot, dtype=np.int16)
    starts = np.zeros(NRB, dtype=np.int64)
    starts[1:] = np.cumsum(cnt_seq)[:-1]
    j_within = np.arange(len(r)) - starts[pos]
    slot = pos * epb + j_within
    col_pad[slot] = c.astype(np.int16)

    # P tiles: logically [ncht, 128, 128] bf16; P[k, p, rl] = w for edge
    # (k*128+p). Device layout groups GCH chunks per gather with partition-
    # major rows so one plain 2D DMA per gather lands them in SBUF:
    # [ngath*128, GCH*128] where row (gi*128+p), cols [q*128:(q+1)*128] is
    # chunk (gi*GCH+q)'s partition-p row.
    p_tiles = np.zeros(ncht * P * P, dtype=BF16)
    chunk = slot // P
    part = slot % P
    p_tiles[chunk * (P * P) + part * P + rl] = w.astype(BF16)
    ngath_p = ncht // GCH
    p_tiles = (p_tiles.reshape(ngath_p, GCH, P, P)
               .transpose(0, 2, 1, 3).reshape(ngath_p * P, GCH * P))

    # gather idx layout: per gather of GCH chunks, wrapped [16, n/16],
    # tiled to 128 partitions; gathers concatenated along free dim
    gpg = GCH * P
    ngath = tot // gpg
    idx_grp = col_pad.reshape(ngath, gpg // 16, 16)
    idx_wrapped = idx_grp.transpose(0, 2, 1)
    idx_dev = np.tile(idx_wrapped, (1, 8, 1))
    idx_dev = np.concatenate(idx_dev, axis=1)
    return {
        "gidx": np.ascontiguousarray(idx_dev),
        "ptiles": np.ascontiguousarray(p_tiles),
    }


def _build_graph(n_hops, nchunk_pb):
    """Build the SPMD Bass graph (identical for all cores)."""
    ncht = NRB * nchunk_pb
    gpg = GCH * P
    nidx_tot = NRB * nchunk_pb * P
    ngath = nidx_tot // gpg
    assert ncht % GCH == 0

    nc = bacc.Bacc("TRN2", target_bir_lowering=False, debug=False,
                   num_devices=N_CORES, num_swdge_queues=4)

    h0_in = nc.dram_tensor("h0", [NB, N_FEAT], mybir.dt.bfloat16,
                           kind="ExternalInput")
    gidx_in = nc.dram_tensor("gidx", [P, nidx_tot // 16], mybir.dt.int16,
                             kind="ExternalInput")
    pt_in = nc.dram_tensor("ptiles", [ngath * P, GCH * P], mybir.dt.bfloat16,
                           kind="ExternalInput")
    y_out = nc.dram_tensor("y", [NB, N_FEAT], mybir.dt.bfloat16,
                           kind="ExternalOutput")

    with tile.TileContext(nc) as tc:
        with tc.tile_pool(name="sbuf", bufs=8) as sbuf, \
             tc.tile_pool(name="sbuf_idx", bufs=1) as sbuf_idx, \
             tc.tile_pool(name="psum", bufs=8, space="PSUM") as psum, \
             tc.tile_pool(name="dram", bufs=2, space="DRAM") as dram:

            h_tabs = [h0_in.ap()[:, :]]
            for hop in range(n_hops):
                partial = dram.tile([NPAD, N_FEAT], mybir.dt.bfloat16,
                                    tag="partial")
                h_tab = h_tabs[hop]
                idx_hop = sbuf_idx.tile([P, nidx_tot // 16], mybir.dt.int16,
                                        tag="idxhop")
                nc.sync.dma_start(idx_hop[:], gidx_in.ap()[:, :])
                g_list = [None] * ngath
                p_list = [None] * ngath

                def issue_gather(gi, h_tab=h_tab, g_list=g_list,
                                 p_list=p_list, idx_hop=idx_hop):
                    g_t = sbuf.tile([P, GCH, N_FEAT],
                                    mybir.dt.bfloat16, tag="gath")
                    nc.gpsimd.dma_gather(
                        out_ap=g_t[:],
                        in_ap=h_tab,
                        idxs_ap=idx_hop[:, gi * (gpg // 16):(gi + 1) * (gpg // 16)],
                        num_idxs=gpg,
                        num_idxs_reg=gpg,
                        elem_size=N_FEAT,
                        queue_num=gi % 4,
                    )
                    g_list[gi] = g_t
                    # P tiles for this gather's GCH chunks, loaded via the
                    # ScalarE HWDGE ring so they don't queue behind the
                    # Sync-ring evict/idx DMAs
                    p_t = sbuf.tile([P, GCH, P], mybir.dt.bfloat16,
                                    tag="ptile")
                    nc.scalar.dma_start(
                        p_t[:], pt_in.ap()[gi * P:(gi + 1) * P, :])
                    p_list[gi] = p_t

                h_next = dram.tile([NB, N_FEAT], mybir.dt.bfloat16,
                                   tag="hnext")
                bpseg = NRB // NSEG        # 112 processed blocks per slab
                rseg = NB // NSEG          # 1792 h_next rows per slab
                rs_next = 0

                def maybe_issue_rs(done_blocks, force=False):
                    nonlocal rs_next
                    while rs_next < NSEG:
                        need = (rs_next + 1) * bpseg + 128
                        if not force and done_blocks < min(need, NRB):
                            break
                        if not force and rs_next == NSEG - 1:
                            break
                        j = rs_next
                        nc.gpsimd.collective_compute(
                            "ReduceScatter",
                            mybir.AluOpType.add,
                            replica_groups=[list(range(N_CORES))],
                            ins=[partial[j * bpseg * P:(j + 1) * bpseg * P,
                                         :].opt()],
                            outs=[h_next[j * rseg:(j + 1) * rseg, :].opt()],
                        )
                        rs_next += 1

                next_rb = 0
                for gi in range(ngath):
                    issue_gather(gi)
                    while (next_rb < NRB
                           and (next_rb + 1) * nchunk_pb <= (gi + 1) * GCH):
                        rbid = next_rb
                        ps = psum.tile([P, N_FEAT], mybir.dt.float32,
                                       space="PSUM", tag="ps")
                        for cch in range(nchunk_pb):
                            k = rbid * nchunk_pb + cch
                            nc.tensor.matmul(
                                out=ps[:],
                                lhsT=p_list[k // GCH][:, k % GCH, :],
                                rhs=g_list[k // GCH][:, k % GCH, :],
                                start=(cch == 0),
                                stop=(cch == nchunk_pb - 1),
                            )
                        ev = sbuf.tile([P, N_FEAT], mybir.dt.bfloat16,
                                       tag="evict")
                        nc.vector.tensor_copy(ev[:], ps[:])
                        nc.sync.dma_start(
                            partial[rbid * P:(rbid + 1) * P, :], ev[:])
                        next_rb += 1
                        maybe_issue_rs(next_rb)
                maybe_issue_rs(NRB, force=True)
                h_tabs.append(h_next[:])

            nc.sync.dma_start(y_out.ap()[:, :], h_tabs[n_hops])

    nc.compile()
    return nc


_GRAPH_CACHE = {}


def kernel(x, weights, row, col, layer_number):
    x = np.asarray(x)
    weights = np.asarray(weights)
    rows = np.asarray(row).astype(np.int64)
    cols = np.asarray(col).astype(np.int64)
    n_hops = int(layer_number)
    if n_hops == 0:
        return x.astype(np.float32)

    preps = [_prep_core(rows, cols, weights, c) for c in range(N_CORES)]
    nchunk_pb = max(int(np.ceil(p[5].max() / P)) for p in preps)
    nchunk_pb = max(nchunk_pb, 1)

    key = (n_hops, nchunk_pb)
    if key not in _GRAPH_CACHE:
        _GRAPH_CACHE[key] = _build_graph(n_hops, nchunk_pb)
    nc = _GRAPH_CACHE[key]

    x_pad = np.zeros((NPAD, N_FEAT), dtype=np.float32)
    x_pad[:N_NODES] = x
    x_bf = x_pad.astype(BF16)

    in_maps = []
    for c in range(N_CORES):
        dev = _pack_core(*preps[c], nchunk_pb)
        in_maps.append({
            "h0": np.ascontiguousarray(x_bf[NB * c:NB * (c + 1)]),
            "gidx": dev["gidx"],
            "ptiles": dev["ptiles"],
        })

    res = run_bass_kernel_spmd(nc, in_maps, core_ids=list(range(N_CORES)))
    y = np.concatenate([res.results[c]["y"].astype(np.float32)
                        for c in range(N_CORES)], axis=0)
    return y[:N_NODES]



# revision 4
# speedup vs baseline: 1.0263x; 1.0237x over previous
"""Trainium2 Bass kernel for nn_AdultConnectome (gnn_message_passing).

Computes y = A^L @ x for a COO sparse adjacency A (100000 nodes, 3.2M edges),
x [100000, 512] fp32, L = layer_number hops.

Distribution: 8 NeuronCores; core c owns the column-node block
[12544*c, 12544*(c+1)) and ALL 512 features (bf16). Edges are partitioned by
their source (col) node block, so every per-edge gather index is block-local
(< 12544, fits the gather DMA's int16 index limit). Each hop:

  1. dma_gather: per edge e, fetch h[col_local[e], :] (512 bf16 = 1KB rows)
     from the core-local table in HBM. Edges are pre-sorted by destination
     row block (and by source within a block, for HBM locality), padded per
     128-row destination block to a uniform chunk count. Gathers are capped
     at 1024 indices (larger single dma_gathers overflow the SWDGE
     descriptor ring) and round-robined over 4 SWDGE queues so descriptor
     generation uses all four Q7 core pairs.
  2. For each 128-edge chunk, build the scatter matrix
     P[e, r] = w[e] * (r == rl[e]) ON-DEVICE with one DVE tensor_scalar
     (iota row vector vs per-partition rl, scaled by per-partition w), and
     accumulate PSUM[r, f] += P^T @ G on TensorE. This is the segment-sum.
     No P-tile HBM traffic.
  3. Evict each 128-row block to a [100352, 512] bf16 partial in HBM.
  4. ReduceScatter(add) over all 8 cores (14 slabs, issued as soon as their
     partial rows are evicted) sums the partials and hands core c its own
     12544-row block for the next hop's gather table.

Gather indices and per-edge (rl, w) metadata are loaded into SBUF once,
before the hop loop (they are hop-invariant). All structure is computed
host-side from the actual edge data and baked into the compiled graph; it
is identical on all 8 cores (SPMD), with per-core differences only in
input tensors.
"""

import numpy as np
import ml_dtypes

import concourse.bass as bass
import concourse.bacc as bacc
import concourse.tile as tile
import concourse.mybir as mybir
from concourse.bass_utils import run_bass_kernel_spmd

BF16 = ml_dtypes.bfloat16

N_CORES = 8
P = 128
N_NODES = 100000
N_FEAT = 512
NB = 12544                 # nodes per core block (100352 = 8 * 12544)
NPAD = NB * N_CORES        # 100352
NRB = NPAD // P            # 784 row blocks
GCH = 8                    # chunks (128 edges each) per gather (1024 idx max)
NSEG = 14                  # ReduceScatter slabs per hop (784 = 14*56)


def _prep_core(rows, cols, ws, core):
    """Per-core edge preprocessing."""
    lo, hi = NB * core, NB * (core + 1)
    m = (cols >= lo) & (cols < hi)
    r = rows[m]
    c = (cols[m] - lo).astype(np.int64)
    w = ws[m]
    # sort by dest row block, then by source within the block so each
    # gather's descriptor addresses are monotonic (HBM locality)
    order = np.lexsort((c, r >> 7))
    r, c, w = r[order], c[order], w[order]
    rb = r >> 7
    rl = (r & 127).astype(np.int64)
    cnt = np.bincount(rb, minlength=NRB)
    return r, c, w, rb, rl, cnt


def _block_seq():
    """Row-block processing order: segment-major (q, core, i) so that each
    of the NSEG ReduceScatter slabs covers a contiguous run of processed
    blocks and can be issued while later segments still compute."""
    bpc = NRB // N_CORES              # 98 blocks per core block
    bps = bpc // NSEG                 # 7 blocks per (segment, core)
    seq = []
    for q in range(NSEG):
        for cc in range(N_CORES):
            for i in range(bps):
                seq.append(cc * bpc + q * bps + i)
    return np.array(seq, dtype=np.int64)


def _pack_core(r, c, w, rb, rl, cnt, nchunk_pb):
    """Pack one core's edges into padded device arrays (idx + rl/w meta)."""
    epb = nchunk_pb * P               # padded edges per row block
    tot = NRB * epb
    ncht = NRB * nchunk_pb
    bs = _block_seq()
    posof = np.empty(NRB, dtype=np.int64)
    posof[bs] = np.arange(NRB)
    order2 = np.argsort(posof[rb], kind="stable")
    r, c, w, rb, rl = r[order2], c[order2], w[order2], rb[order2], rl[order2]
    pos = posof[rb]
    cnt_seq = cnt[bs]
    col_pad = np.zeros(tot, dtype=np.int16)
    starts = np.zeros(NRB, dtype=np.int64)
    starts[1:] = np.cumsum(cnt_seq)[:-1]
    j_within = np.arange(len(r)) - starts[pos]
    slot = pos * epb + j_within
    col_pad[slot] = c.astype(np.int16)

    # per-slot metadata: rl (dest row within block) and w, fp32, laid out
    # [128 partitions, ncht chunks]; padded slots have w=0 (and idx 0, a
    # valid row, so the gathered garbage is finite and multiplied by 0)
    chunk = slot // P
    part = slot % P
    rl_arr = np.zeros((P, ncht), dtype=np.float32)
    w_arr = np.zeros((P, ncht), dtype=np.float32)
    rl_arr[part, chunk] = rl.astype(np.float32)
    w_arr[part, chunk] = w.astype(np.float32)
    rlw = np.concatenate([rl_arr, w_arr], axis=1)

    # gather idx layout: per gather of GCH chunks, wrapped [16, n/16],
    # tiled to 128 partitions; gathers concatenated along free dim
    gpg = GCH * P
    ngath = tot // gpg
    idx_grp = col_pad.reshape(ngath, gpg // 16, 16)
    idx_wrapped = idx_grp.transpose(0, 2, 1)
    idx_dev = np.tile(idx_wrapped, (1, 8, 1))
    idx_dev = np.concatenate(idx_dev, axis=1)
    return {
        "gidx": np.ascontiguousarray(idx_dev),
        "rlw": np.ascontiguousarray(rlw),
    }


def _build_graph(n_hops, nchunk_pb):
    """Build the SPMD Bass graph (identical for all cores)."""
    ncht = NRB * nchunk_pb
    gpg = GCH * P
    nidx_tot = NRB * nchunk_pb * P
    ngath = nidx_tot // gpg
    assert ncht % GCH == 0

    nc = bacc.Bacc("TRN2", target_bir_lowering=False, debug=False,
                   num_devices=N_CORES, num_swdge_queues=4)

    h0_in = nc.dram_tensor("h0", [NB, N_FEAT], mybir.dt.bfloat16,
                           kind="ExternalInput")
    gidx_in = nc.dram_tensor("gidx", [P, nidx_tot // 16], mybir.dt.int16,
                             kind="ExternalInput")
    rlw_in = nc.dram_tensor("rlw", [P, 2 * ncht], mybir.dt.float32,
                            kind="ExternalInput")
    y_out = nc.dram_tensor("y", [NB, N_FEAT], mybir.dt.bfloat16,
                           kind="ExternalOutput")

    with tile.TileContext(nc) as tc:
        with tc.tile_pool(name="sbuf", bufs=6) as sbuf, \
             tc.tile_pool(name="sbuf_p", bufs=8) as sbuf_p, \
             tc.tile_pool(name="sbuf_e", bufs=6) as sbuf_e, \
             tc.tile_pool(name="sbuf_c", bufs=1) as sbuf_c, \
             tc.tile_pool(name="psum", bufs=8, space="PSUM") as psum, \
             tc.tile_pool(name="dram", bufs=2, space="DRAM") as dram:

            # hop-invariant SBUF state, loaded once
            idx_all = sbuf_c.tile([P, nidx_tot // 16], mybir.dt.int16,
                                  tag="idx")
            nc.sync.dma_start(idx_all[:], gidx_in.ap()[:, :])
            rlw_sb = sbuf_c.tile([P, 2 * ncht], mybir.dt.float32, tag="rlw")
            nc.sync.dma_start(rlw_sb[:], rlw_in.ap()[:, :])
            iota_i = sbuf_c.tile([P, P], mybir.dt.int16, tag="ioi")
            nc.gpsimd.iota(iota_i[:], pattern=[[1, P]], base=0,
                           channel_multiplier=0)
            iota_bf = sbuf_c.tile([P, P], mybir.dt.bfloat16, tag="iob")
            nc.vector.tensor_copy(iota_bf[:], iota_i[:])

            h_tabs = [h0_in.ap()[:, :]]
            for hop in range(n_hops):
                partial = dram.tile([NPAD, N_FEAT], mybir.dt.bfloat16,
                                    tag="partial")
                h_tab = h_tabs[hop]
                g_list = [None] * ngath

                def issue_gather(gi, h_tab=h_tab, g_list=g_list,
                                 idx_all=idx_all):
                    g_t = sbuf.tile([P, GCH, N_FEAT],
                                    mybir.dt.bfloat16, tag="gath")
                    nc.gpsimd.dma_gather(
                        out_ap=g_t[:],
                        in_ap=h_tab,
                        idxs_ap=idx_all[:, gi * (gpg // 16):(gi + 1) * (gpg // 16)],
                        num_idxs=gpg,
                        num_idxs_reg=gpg,
                        elem_size=N_FEAT,
                        queue_num=gi % 4,
                    )
                    g_list[gi] = g_t

                h_next = dram.tile([NB, N_FEAT], mybir.dt.bfloat16,
                                   tag="hnext")
                bpseg = NRB // NSEG        # 56 processed blocks per slab
                rseg = NB // NSEG          # 896 h_next rows per slab
                rs_next = 0

                def maybe_issue_rs(done_blocks, force=False):
                    nonlocal rs_next
                    while rs_next < NSEG:
                        need = (rs_next + 1) * bpseg + 128
                        if not force and done_blocks < min(need, NRB):
                            break
                        if not force and rs_next == NSEG - 1:
                            break
                        j = rs_next
                        nc.gpsimd.collective_compute(
                            "ReduceScatter",
                            mybir.AluOpType.add,
                            replica_groups=[list(range(N_CORES))],
                            ins=[partial[j * bpseg * P:(j + 1) * bpseg * P,
                                         :].opt()],
                            outs=[h_next[j * rseg:(j + 1) * rseg, :].opt()],
                        )
                        rs_next += 1

                next_rb = 0
                for gi in range(ngath):
                    issue_gather(gi)
                    while (next_rb < NRB
                           and (next_rb + 1) * nchunk_pb <= (gi + 1) * GCH):
                        rbid = next_rb
                        ps = psum.tile([P, N_FEAT], mybir.dt.float32,
                                       space="PSUM", tag="ps")
                        for cch in range(nchunk_pb):
                            k = rbid * nchunk_pb + cch
                            p_t = sbuf_p.tile([P, P], mybir.dt.bfloat16,
                                              tag="ptile")
                            nc.vector.tensor_scalar(
                                out=p_t[:], in0=iota_bf[:],
                                scalar1=rlw_sb[:, k:k + 1],
                                scalar2=rlw_sb[:, ncht + k:ncht + k + 1],
                                op0=mybir.AluOpType.is_equal,
                                op1=mybir.AluOpType.mult)
                            nc.tensor.matmul(
                                out=ps[:],
                                lhsT=p_t[:],
                                rhs=g_list[k // GCH][:, k % GCH, :],
                                start=(cch == 0),
                                stop=(cch == nchunk_pb - 1),
                            )
                        ev = sbuf_e.tile([P, N_FEAT], mybir.dt.bfloat16,
                                         tag="evict")
                        nc.vector.tensor_copy(ev[:], ps[:])
                        nc.sync.dma_start(
                            partial[rbid * P:(rbid + 1) * P, :], ev[:])
                        next_rb += 1
                        maybe_issue_rs(next_rb)
                maybe_issue_rs(NRB, force=True)
                h_tabs.append(h_next[:])

            nc.sync.dma_start(y_out.ap()[:, :], h_tabs[n_hops])

    nc.compile()
    return nc


_GRAPH_CACHE = {}


def kernel(x, weights, row, col, layer_number):
    x = np.asarray(x)
    weights = np.asarray(weights)
    rows = np.asarray(row).astype(np.int64)
    cols = np.asarray(col).astype(np.int64)
    n_hops = int(layer_number)
    if n_hops == 0:
        return x.astype(np.float32)

    preps = [_prep_core(rows, cols, weights, c) for c in range(N_CORES)]
    nchunk_pb = max(int(np.ceil(p[5].max() / P)) for p in preps)
    nchunk_pb = max(nchunk_pb, 1)

    key = (n_hops, nchunk_pb)
    if key not in _GRAPH_CACHE:
        _GRAPH_CACHE[key] = _build_graph(n_hops, nchunk_pb)
    nc = _GRAPH_CACHE[key]

    x_pad = np.zeros((NPAD, N_FEAT), dtype=np.float32)
    x_pad[:N_NODES] = x
    x_bf = x_pad.astype(BF16)

    in_maps = []
    for c in range(N_CORES):
        dev = _pack_core(*preps[c], nchunk_pb)
        in_maps.append({
            "h0": np.ascontiguousarray(x_bf[NB * c:NB * (c + 1)]),
            "gidx": dev["gidx"],
            "rlw": dev["rlw"],
        })

    res = run_bass_kernel_spmd(nc, in_maps, core_ids=list(range(N_CORES)))
    y = np.concatenate([res.results[c]["y"].astype(np.float32)
                        for c in range(N_CORES)], axis=0)
    return y[:N_NODES]


# revision 6
# speedup vs baseline: 2.3770x; 2.3161x over previous
"""Trainium2 Bass kernel for nn_AdultConnectome (gnn_message_passing).

Computes y = A^L @ x for a COO sparse adjacency A (100000 nodes, 3.2M edges),
x [100000, 512] fp32, L = layer_number hops.

Distribution (destination-sharded): 8 NeuronCores; core c owns the OUTPUT
row block [12544*c, 12544*(c+1)) and processes exactly the edges whose
destination falls in its block, so the segment-sum per output row is fully
local (no cross-core reduction). The full h table [100352, 512] bf16 lives
in ONE shared HBM buffer (addr_space="Shared") rebuilt each hop by an
AllGather of the per-core 12544-row blocks.

Per hop:
  1. dma_gather: per edge e, fetch h[col[e], :] (512 bf16 = 1KB rows) from
     the shared table. The int16 gather-index limit (< 32768) is handled by
     splitting the table into 4 quarter-views of 25088 rows; edges are
     sorted per destination row block by (source quarter, source), and each
     (block, quarter) run is padded to a uniform Q chunks of 128 so every
     gather reads a single quarter. Gathers are capped at 1024 indices and
     round-robined over 4 SWDGE queues (4 Q7 descriptor-gen core pairs).
  2. Per 128-row destination block, build ALL the block's scatter matrices
     P[e, r] = w[e] * (r == rl[e]) with TWO DVE tensor_tensor ops over a
     [128, 4*Q, 128] tile (iota vs stride-0-broadcast rl, then * w), and
     accumulate PSUM[r, f] += P_chunk^T @ G_chunk on TensorE (4*Q matmuls).
  3. Evict the block to a local h_next [12544, 512] bf16.
  4. AllGather h_next into the next hop's shared table (skipped after the
     final hop; the last h_next is the core's output block).

Gather indices and per-edge (rl, w) metadata are hop-invariant: rl/w load
into SBUF once; indices stream per block. All structure is computed
host-side from the actual edge data and baked into the SPMD graph (identical
on all 8 cores; per-core differences only in input tensors).
"""

import numpy as np
import ml_dtypes

import concourse.bass as bass
import concourse.bacc as bacc
import concourse.tile as tile
import concourse.mybir as mybir
from concourse.bass_utils import run_bass_kernel_spmd

BF16 = ml_dtypes.bfloat16

N_CORES = 8
P = 128
N_NODES = 100000
N_FEAT = 512
NB = 12544                 # output rows per core (100352 = 8 * 12544)
NPAD = NB * N_CORES        # 100352
NBL = NB // P              # 98 dest row blocks per core
NQT = 4                    # quarter tables (int16 idx limit)
NQ = NPAD // NQT           # 25088 rows per quarter


def _prep_core(rows, cols, ws, core):
    """Per-core edge preprocessing (destination sharding)."""
    lo, hi = NB * core, NB * (core + 1)
    m = (rows >= lo) & (rows < hi)
    r = rows[m] - lo
    c = cols[m]
    w = ws[m]
    rb = r >> 7
    q = c // NQ
    # sort by (dest block, source quarter, source)
    order = np.lexsort((c, q, rb))
    r, c, w, rb, q = r[order], c[order], w[order], rb[order], q[order]
    rl = (r & 127).astype(np.int64)
    cq = (c - q * NQ).astype(np.int64)
    cnt = np.bincount(rb * NQT + q, minlength=NBL * NQT).reshape(NBL, NQT)
    return rl, cq, w, rb, q, cnt


def _gather_sizes(Q):
    """Split Q chunks into gathers of at most 8 chunks (1024 idx)."""
    sizes = [8] * (Q // 8)
    if Q % 8:
        sizes.append(Q % 8)
    return sizes


def _pack_core(rl, cq, w, rb, q, cnt, Q):
    """Pack one core's edges into device arrays (wrapped idx + rl/w meta)."""
    ncht = NBL * NQT * Q
    nslots = ncht * P
    run = Q * P                    # slots per (block, quarter) run
    starts = np.zeros(NBL * NQT, dtype=np.int64)
    starts[1:] = np.cumsum(cnt.reshape(-1))[:-1]
    runid = rb * NQT + q
    j_within = np.arange(len(rl)) - starts[runid]
    slot = runid * run + j_within

    idx_flat = np.zeros(nslots, dtype=np.int16)
    idx_flat[slot] = cq.astype(np.int16)
    chunk = slot // P
    part = slot % P
    rl_arr = np.zeros((P, ncht), dtype=np.float32)
    w_arr = np.zeros((P, ncht), dtype=np.float32)
    rl_arr[part, chunk] = rl.astype(np.float32)
    w_arr[part, chunk] = w.astype(np.float32)
    rlw = np.concatenate([rl_arr, w_arr], axis=1)

    # wrapped idx per gather: [16, L/16] replicated to 128 partitions,
    # gathers concatenated along the free dim
    sizes = _gather_sizes(Q)
    cols = []
    off = 0
    for b in range(NBL):
        for qq in range(NQT):
            base = (b * NQT + qq) * run
            o = 0
            for s in sizes:
                L = s * P
                sl = idx_flat[base + o: base + o + L]
                wrapped = sl.reshape(L // 16, 16).T       # [16, L/16]
                cols.append(np.tile(wrapped, (8, 1)))     # [128, L/16]
                o += L
    idx_dev = np.concatenate(cols, axis=1)
    assert idx_dev.shape == (P, nslots // 16)
    return {
        "gidx": np.ascontiguousarray(idx_dev),
        "rlw": np.ascontiguousarray(rlw),
    }


def _build_graph(n_hops, Q):
    """Build the SPMD Bass graph (identical for all cores)."""
    ncht = NBL * NQT * Q
    nslots = ncht * P
    sizes = _gather_sizes(Q)
    icols_pg = [s * P // 16 for s in sizes]   # idx cols per gather
    icols_pq = Q * P // 16                    # idx cols per (block, quarter)
    icols_pb = NQT * icols_pq                 # idx cols per block

    nc = bacc.Bacc("TRN2", target_bir_lowering=False, debug=False,
                   num_devices=N_CORES, num_swdge_queues=4)

    h0_in = nc.dram_tensor("h0", [NB, N_FEAT], mybir.dt.bfloat16,
                           kind="ExternalInput")
    gidx_in = nc.dram_tensor("gidx", [P, nslots // 16], mybir.dt.int16,
                             kind="ExternalInput")
    rlw_in = nc.dram_tensor("rlw", [P, 2 * ncht], mybir.dt.float32,
                            kind="ExternalInput")
    y_out = nc.dram_tensor("y", [NB, N_FEAT], mybir.dt.bfloat16,
                           kind="ExternalOutput")

    with tile.TileContext(nc) as tc:
        with tc.tile_pool(name="sbuf_g", bufs=2) as sbuf_g, \
             tc.tile_pool(name="sbuf_p", bufs=2) as sbuf_p, \
             tc.tile_pool(name="sbuf_e", bufs=4) as sbuf_e, \
             tc.tile_pool(name="sbuf_i", bufs=4) as sbuf_i, \
             tc.tile_pool(name="sbuf_c", bufs=1) as sbuf_c, \
             tc.tile_pool(name="psum", bufs=8, space="PSUM") as psum, \
             tc.tile_pool(name="dram", bufs=2, space="DRAM") as dram:

            # hop-invariant SBUF state
            rlw_sb = sbuf_c.tile([P, 2 * ncht], mybir.dt.float32, tag="rlw")
            nc.sync.dma_start(rlw_sb[:], rlw_in.ap()[:, :])
            iota_f = sbuf_c.tile([P, NQT * Q, P], mybir.dt.float32,
                                 tag="iof")
            nc.gpsimd.iota(iota_f[:], pattern=[[0, NQT * Q], [1, P]],
                           base=0, channel_multiplier=0,
                           allow_small_or_imprecise_dtypes=True)

            # initial shared table from the per-core input blocks
            h0_loc = dram.tile([NB, N_FEAT], mybir.dt.bfloat16, tag="h0l",
                               bufs=1)
            nc.sync.dma_start(h0_loc[:, :], h0_in.ap()[:, :])
            tab = dram.tile([NPAD, N_FEAT], mybir.dt.bfloat16, tag="tab",
                            addr_space="Shared")
            nc.gpsimd.collective_compute(
                "AllGather", mybir.AluOpType.bypass,
                replica_groups=[list(range(N_CORES))],
                ins=[h0_loc[:, :].opt()],
                outs=[tab[:, :].opt()],
            )

            gq = 0  # gather queue round-robin

            for hop in range(n_hops):
                hnl = dram.tile([NB, N_FEAT], mybir.dt.bfloat16, tag="hnl")

                for b in range(NBL):
                    idx_t = sbuf_i.tile([P, icols_pb], mybir.dt.int16,
                                        tag="idx")
                    nc.scalar.dma_start(
                        idx_t[:],
                        gidx_in.ap()[:, b * icols_pb:(b + 1) * icols_pb])

                    g_tiles = []
                    for qq in range(NQT):
                        tq = tab[NQ * qq:NQ * (qq + 1), :]
                        parts = []
                        o = 0
                        for si, s in enumerate(sizes):
                            g_t = sbuf_g.tile([P, s, N_FEAT],
                                              mybir.dt.bfloat16,
                                              tag=f"g{qq}_{si}")
                            ic0 = qq * icols_pq + o
                            nc.gpsimd.dma_gather(
                                out_ap=g_t[:],
                                in_ap=tq,
                                idxs_ap=idx_t[:, ic0:ic0 + icols_pg[si]],
                                num_idxs=s * P,
                                num_idxs_reg=s * P,
                                elem_size=N_FEAT,
                                queue_num=gq % 4,
                            )
                            gq += 1
                            parts.append((g_t, s))
                            o += icols_pg[si]
                        g_tiles.append(parts)

                    # all P tiles for the block in two DVE ops
                    p_blk = sbuf_p.tile([P, NQT * Q, P], mybir.dt.bfloat16,
                                        tag="pblk")
                    k0 = b * NQT * Q
                    rl_b = rlw_sb[:, k0:k0 + NQT * Q].unsqueeze(2) \
                        .broadcast_to([P, NQT * Q, P])
                    w_b = rlw_sb[:, ncht + k0:ncht + k0 + NQT * Q] \
                        .unsqueeze(2).broadcast_to([P, NQT * Q, P])
                    nc.vector.tensor_tensor(out=p_blk[:], in0=iota_f[:],
                                            in1=rl_b,
                                            op=mybir.AluOpType.is_equal)
                    nc.vector.tensor_tensor(out=p_blk[:], in0=p_blk[:],
                                            in1=w_b,
                                            op=mybir.AluOpType.mult)

                    ps = psum.tile([P, N_FEAT], mybir.dt.float32,
                                   space="PSUM", tag="ps")
                    k = 0
                    for qq in range(NQT):
                        for (g_t, s) in g_tiles[qq]:
                            for cc in range(s):
                                nc.tensor.matmul(
                                    out=ps[:],
                                    lhsT=p_blk[:, k, :],
                                    rhs=g_t[:, cc, :],
                                    start=(k == 0),
                                    stop=(k == NQT * Q - 1),
                                )
                                k += 1
                    ev = sbuf_e.tile([P, N_FEAT], mybir.dt.bfloat16,
                                     tag="evict")
                    nc.vector.tensor_copy(ev[:], ps[:])
                    nc.sync.dma_start(hnl[b * P:(b + 1) * P, :], ev[:])

                if hop < n_hops - 1:
                    tab = dram.tile([NPAD, N_FEAT], mybir.dt.bfloat16,
                                    tag="tab", addr_space="Shared")
                    nc.gpsimd.collective_compute(
                        "AllGather", mybir.AluOpType.bypass,
                        replica_groups=[list(range(N_CORES))],
                        ins=[hnl[:, :].opt()],
                        outs=[tab[:, :].opt()],
                    )

            nc.sync.dma_start(y_out.ap()[:, :], hnl[:, :])

    nc.compile()
    return nc


_GRAPH_CACHE = {}


def kernel(x, weights, row, col, layer_number):
    x = np.asarray(x)
    weights = np.asarray(weights)
    rows = np.asarray(row).astype(np.int64)
    cols = np.asarray(col).astype(np.int64)
    n_hops = int(layer_number)
    if n_hops == 0:
        return x.astype(np.float32)

    preps = [_prep_core(rows, cols, weights, c) for c in range(N_CORES)]
    Q = max(int(np.ceil(p[5].max() / P)) for p in preps)
    Q = max(Q, 1)

    key = (n_hops, Q)
    if key not in _GRAPH_CACHE:
        _GRAPH_CACHE[key] = _build_graph(n_hops, Q)
    nc = _GRAPH_CACHE[key]

    x_pad = np.zeros((NPAD, N_FEAT), dtype=np.float32)
    x_pad[:N_NODES] = x
    x_bf = x_pad.astype(BF16)

    in_maps = []
    for c in range(N_CORES):
        dev = _pack_core(*preps[c], Q)
        in_maps.append({
            "h0": np.ascontiguousarray(x_bf[NB * c:NB * (c + 1)]),
            "gidx": dev["gidx"],
            "rlw": dev["rlw"],
        })

    res = run_bass_kernel_spmd(nc, in_maps, core_ids=list(range(N_CORES)))
    y = np.concatenate([res.results[c]["y"].astype(np.float32)
                        for c in range(N_CORES)], axis=0)
    return y[:N_NODES]
